# revision 3
# baseline (speedup 1.0000x reference)
"""Trainium2 Bass kernel for nn_DepthGlobalPool (histogram_binning).

Math: out[b,:,h,w] = means[bin(b,h,w)] where
  bin = histogram bin of depth over global [min,max], 10 equal bins
  means[n] = mean over pixels p in bin n of (W @ features[p] + bias)
Because the 1x1 conv is linear, the per-bin sums of conv outputs equal
W @ (per-bin sums of features) + count*bias, so the per-pixel conv never
needs to be materialized:
  G[n, o]  = sum_{p in bin n} (W @ features[p])[o]      (device, phase A)
  means    = G_global / max(counts,1) + bias*(counts>0) (host, tiny)
  out[p]   = means[bin(p)]                              (device, phase B)

Distribution: data-parallel over batch B (2 batches per core on 8 cores).
Phase A produces per-core partial G [10,64]; the tiny partials are reduced
on host between the two NEFF launches (cheaper + more deterministic than an
on-device AllReduce, which measured 35-70us of latency+skew).

Phase A (per core): for each 128-pixel block, matmul with the feature block
as the STATIONARY operand (lhsT=[128c,128p], rhs=W^T[128c,64]) produces the
conv output transposed, g_T[128p,64], in PSUM -- this puts pixels on
partitions so a second matmul (lhsT=onehot_T[128p,10], rhs=g_T) can contract
over pixels, accumulating G[10,64] in PSUM across all blocks.

Phase B (per core): out tile [64,512] = means^T @ onehot[10,512] with the
means as stationary; means are split hi/lo into two bf16 matrices so two
accumulating bf16 matmuls reproduce fp32-accurate means (one-hot is exact
in bf16).
"""

import os
import numpy as np
import ml_dtypes

import concourse.bass as bass  # noqa: F401  (registers types)
import concourse.tile as tile
import concourse.bass_utils as bass_utils
from concourse import bacc, mybir

# Problem shape (hardcoded per contract)
B, CIN, COUT, H, W_ = 16, 128, 64, 192, 192
HW = H * W_                      # 36864
NB = 10                          # histogram bins
N_CORES = 8
BPC = B // N_CORES               # batches per core = 2
PPC = BPC * HW                   # pixels per core = 73728
BLK = 128                        # pixels per feature block (matmul stationary)
GROUP_PX = 1024                  # pixels per PSUM group = 8 blocks * 128
BLK_PER_GROUP = GROUP_PX // BLK  # 8
SLAB_PX = 4096                   # pixels per feature DMA slab
N_SLABS = PPC // SLAB_PX         # 18
GROUPS_PER_SLAB = SLAB_PX // GROUP_PX  # 4
N_GROUPS = PPC // GROUP_PX       # 72
N_BLOCKS = PPC // BLK            # 576
OHA_STRIDE = 16                  # onehot_T block stride (padded 10 -> 16)

BF16 = mybir.dt.bfloat16
F32 = mybir.dt.float32

_CACHE = {}

# exec times (ns) of the last kernel() call, per NEFF, when tracing enabled
LAST_EXEC_NS = {}


def _install_ntff_hook():
    """Optionally enable NTFF profiling under axon (agent image lacks
    antenv.axon_hooks). Best-effort; harmless if unavailable."""
    import sys, types
    if "antenv.axon_hooks" in sys.modules:
        return True
    try:
        mod = types.ModuleType("antenv.axon_hooks")
        _hook = [None]
        mod.set_axon_ntff_profile_hook = lambda h: _hook.__setitem__(0, h)
        mod.get_axon_ntff_profile_hook = lambda: _hook[0]
        import antenv
        from trn_agent_boot.trn_boot import _ntff_profile_via_ctypes
        antenv.axon_hooks = mod
        sys.modules["antenv.axon_hooks"] = mod
        mod.set_axon_ntff_profile_hook(
            _ntff_profile_via_ctypes("/opt/axon/libaxon_pjrt.so"))
        return True
    except Exception:
        return False


def _build_neff_a():
    """Phase A: per-core partial per-bin sums of conv outputs, G[10,64]."""
    nc = bacc.Bacc("TRN2", target_bir_lowering=False, debug=False,
                   enable_asserts=True, num_devices=N_CORES)
    feats_t = nc.dram_tensor("feats", [BPC, CIN, HW], F32, kind="ExternalInput")
    oha_t = nc.dram_tensor("oha", [128, N_BLOCKS * OHA_STRIDE], BF16,
                           kind="ExternalInput")
    wt_t = nc.dram_tensor("wt", [CIN, COUT], BF16, kind="ExternalInput")
    gpart_t = nc.dram_tensor("gpart", [NB, COUT], F32, kind="ExternalOutput")

    feats = feats_t.ap()
    with tile.TileContext(nc) as tc:
        with tc.tile_pool(name="cst", bufs=1) as cst, \
             tc.tile_pool(name="fpool", bufs=3) as fpool, \
             tc.tile_pool(name="gpool", bufs=3) as gpool, \
             tc.tile_pool(name="spool", bufs=1) as spool, \
             tc.tile_pool(name="pconv", bufs=3, space="PSUM") as pconv, \
             tc.tile_pool(name="pg", bufs=1, space="PSUM") as pg:

            wt_s = cst.tile([CIN, COUT], BF16)
            nc.sync.dma_start(wt_s[:], wt_t.ap()[:])
            oha_s = cst.tile([128, N_BLOCKS * OHA_STRIDE], BF16)
            nc.sync.dma_start(oha_s[:], oha_t.ap()[:])

            G_ps = pg.tile([NB, COUT], F32, space="PSUM")

            gi = 0
            for s in range(N_SLABS):
                px0 = s * SLAB_PX
                b, o = px0 // HW, px0 % HW
                fs = fpool.tile([CIN, SLAB_PX], BF16)
                # SWDGE cast f32 -> bf16 during the DMA
                nc.gpsimd.dma_start(fs[:], feats[b, :, o:o + SLAB_PX])
                for g in range(GROUPS_PER_SLAB):
                    ps = pconv.tile([128, 8 * COUT], F32, space="PSUM")
                    for j in range(BLK_PER_GROUP):
                        f0 = g * GROUP_PX + j * BLK
                        nc.tensor.matmul(
                            ps[:, COUT * j:COUT * (j + 1)],
                            lhsT=fs[:, f0:f0 + BLK],
                            rhs=wt_s[:],
                            start=True, stop=True)
                    gsb = gpool.tile([128, 8 * COUT], BF16)
                    if gi % 2 == 0:
                        nc.vector.tensor_copy(gsb[:], ps[:])
                    else:
                        nc.scalar.copy(gsb[:], ps[:])
                    for j in range(BLK_PER_GROUP):
                        blk = gi * BLK_PER_GROUP + j
                        nc.tensor.matmul(
                            G_ps[:],
                            lhsT=oha_s[:, blk * OHA_STRIDE:blk * OHA_STRIDE + NB],
                            rhs=gsb[:, COUT * j:COUT * (j + 1)],
                            start=(blk == 0), stop=(blk == N_BLOCKS - 1))
                    gi += 1

            g_out = spool.tile([NB, COUT], F32)
            nc.vector.tensor_copy(g_out[:], G_ps[:])
            nc.sync.dma_start(gpart_t.ap()[:], g_out[:])
    nc.compile()
    return nc


def _build_neff_b():
    """Phase B: out[b,:,p] = means[bin(p)] via means-stationary matmuls."""
    nc = bacc.Bacc("TRN2", target_bir_lowering=False, debug=False,
                   enable_asserts=True, num_devices=N_CORES)
    mh_t = nc.dram_tensor("mh", [NB, COUT], BF16, kind="ExternalInput")
    ml_t = nc.dram_tensor("ml", [NB, COUT], BF16, kind="ExternalInput")
    ohb_t = nc.dram_tensor("ohb", [NB, PPC], BF16, kind="ExternalInput")
    out_t = nc.dram_tensor("out", [BPC, COUT, HW], F32, kind="ExternalOutput")

    OH_SLAB = 8192               # one-hot pixels per DMA slab
    CH = 1024                    # output pixels per PSUM chunk

    out_ap = out_t.ap()
    ohb = ohb_t.ap()
    with tile.TileContext(nc) as tc:
        with tc.tile_pool(name="cst", bufs=1) as cst, \
             tc.tile_pool(name="ohpool", bufs=3) as ohpool, \
             tc.tile_pool(name="opool", bufs=3) as opool, \
             tc.tile_pool(name="pout", bufs=3, space="PSUM") as pout:

            mh_s = cst.tile([NB, COUT], BF16)
            nc.sync.dma_start(mh_s[:], mh_t.ap()[:])
            ml_s = cst.tile([NB, COUT], BF16)
            nc.sync.dma_start(ml_s[:], ml_t.ap()[:])

            ci = 0
            for s in range(PPC // OH_SLAB):
                oh_s = ohpool.tile([NB, OH_SLAB], BF16)
                nc.sync.dma_start(oh_s[:], ohb[:, s * OH_SLAB:(s + 1) * OH_SLAB])
                for u in range(OH_SLAB // CH):
                    px0 = s * OH_SLAB + u * CH
                    b, o = px0 // HW, px0 % HW
                    po = pout.tile([128, 512], F32, space="PSUM")
                    for half, colbase in ((0, 0), (1, 64)):
                        rhs = oh_s[:, u * CH + half * 512:u * CH + half * 512 + 512]
                        sl = po[colbase:colbase + 64, :]
                        nc.tensor.matmul(sl, lhsT=mh_s[:], rhs=rhs,
                                         start=True, stop=False,
                                         tile_position=(0, colbase))
                        nc.tensor.matmul(sl, lhsT=ml_s[:], rhs=rhs,
                                         start=False, stop=True,
                                         tile_position=(0, colbase))
                    ob = opool.tile([128, 512], F32)
                    if ci % 2 == 0:
                        nc.vector.tensor_copy(ob[:], po[:])
                    else:
                        nc.scalar.copy(ob[:], po[:])
                    # DRAM side is 3-D (partition q of ob pairs with (i=q//64,
                    # c=q%64)); SBUF side must stay a plain [128,512] AP --
                    # splitting the SBUF partition dim produces a bogus AP.
                    dst = out_ap[b, :, o:o + CH].rearrange("c (i p) -> i c p", i=2)
                    nc.sync.dma_start(dst, ob[:])
                    ci += 1
    nc.compile()
    return nc


def _get_modules():
    if "a" not in _CACHE:
        _CACHE["a"] = _build_neff_a()
        _CACHE["b"] = _build_neff_b()
    return _CACHE["a"], _CACHE["b"]


def kernel(features, depth, weight, bias, depthpool=None):
    trace = bool(int(os.environ.get("KERNEL_TRACE", "0")))
    if trace:
        trace = _install_ntff_hook()

    features = np.asarray(features, dtype=np.float32)
    depth = np.asarray(depth, dtype=np.float32)
    weight = np.asarray(weight, dtype=np.float32)
    bias = np.asarray(bias, dtype=np.float32)

    # ---- host: histogram binning of depth (exact f32 replica of reference)
    d = depth[:, 0]                                     # [B, H, W] f32
    dmin, dmax = d.min(), d.max()
    width = np.float32((dmax - dmin) / np.float32(NB))
    bins = np.clip(np.floor((d - dmin) / width).astype(np.int32), 0, NB - 1)
    bins = bins.reshape(B, HW)
    counts = np.bincount(bins.ravel(), minlength=NB).astype(np.float64)

    arange_nb = np.arange(NB, dtype=np.int32)
    wt_bf = np.ascontiguousarray(weight.T).astype(ml_dtypes.bfloat16)  # [128,64]

    in_maps_a = []
    in_maps_b_onehot = []
    for c in range(N_CORES):
        binsc = bins[BPC * c:BPC * (c + 1)].reshape(PPC)       # [73728]
        # onehot_T, padded: oha[p, blk*16 + n] = (binsc[blk*128+p] == n)
        bb = binsc.reshape(N_BLOCKS, BLK)                       # [576, 128]
        oha = np.zeros((128, N_BLOCKS, OHA_STRIDE), dtype=ml_dtypes.bfloat16)
        oha[:, :, :NB] = (bb.T[:, :, None] == arange_nb).astype(ml_dtypes.bfloat16)
        oha = np.ascontiguousarray(oha.reshape(128, N_BLOCKS * OHA_STRIDE))
        # onehot rows: ohb[n, p] = (binsc[p] == n)
        ohb = (arange_nb[:, None] == binsc[None, :]).astype(ml_dtypes.bfloat16)
        feats_c = features[BPC * c:BPC * (c + 1)].reshape(BPC, CIN, HW)
        in_maps_a.append({"feats": feats_c, "oha": oha, "wt": wt_bf})
        in_maps_b_onehot.append(ohb)

    nc_a, nc_b = _get_modules()
    core_ids = list(range(N_CORES))

    res_a = bass_utils.run_bass_kernel_spmd(nc_a, in_maps_a, core_ids=core_ids,
                                            trace=trace)
    if trace:
        LAST_EXEC_NS["A"] = res_a.exec_time_ns

    G = np.zeros((NB, COUT), dtype=np.float64)
    for c in range(N_CORES):
        G += res_a.results[c]["gpart"].astype(np.float64)

    means = G / np.maximum(counts, 1.0)[:, None] \
        + bias.astype(np.float64)[None, :] * (counts > 0)[:, None]
    means = means.astype(np.float32)
    mh = means.astype(ml_dtypes.bfloat16)
    ml = (means - mh.astype(np.float32)).astype(ml_dtypes.bfloat16)

    in_maps_b = [{"mh": mh, "ml": ml, "ohb": in_maps_b_onehot[c]}
                 for c in range(N_CORES)]
    res_b = bass_utils.run_bass_kernel_spmd(nc_b, in_maps_b, core_ids=core_ids,
                                            trace=trace)
    if trace:
        LAST_EXEC_NS["B"] = res_b.exec_time_ns

    out = np.empty((B, COUT, H, W_), dtype=np.float32)
    for c in range(N_CORES):
        out[BPC * c:BPC * (c + 1)] = \
            res_b.results[c]["out"].reshape(BPC, COUT, H, W_)
    return out


# revision 6
# speedup vs baseline: 1.4007x; 1.4007x over previous
"""Trainium2 Bass kernel for nn_DepthGlobalPool (histogram_binning).

Math: out[b,:,h,w] = means[bin(b,h,w)] where
  bin = histogram bin of depth over global [min,max], 10 equal bins
  means[n] = mean over pixels p in bin n of (W @ features[p] + bias)
Because the 1x1 conv is linear, the per-bin sums of conv outputs equal
W @ (per-bin sums of features) + count*bias, so the per-pixel conv never
needs to be materialized:
  G[n, o]  = sum_{p in bin n} (W @ features[p])[o]      (device, phase A)
  means    = G_global / max(counts,1) + bias*(counts>0) (host, tiny)
  out[p]   = means[bin(p)]                              (device, phase B)

Distribution: data-parallel over batch B (2 batches per core on 8 cores).
Phase A produces per-core partial G [10,64]; the tiny partials are reduced
on host between the two NEFF launches (cheaper + more deterministic than an
on-device AllReduce, which measured 35-70us of latency+skew).

Phase A (per core): for each 128-pixel block, matmul with the feature block
as the STATIONARY operand (lhsT=[128c,128p], rhs=W^T[128c,64]) produces the
conv output transposed, g_T[128p,64], in PSUM -- this puts pixels on
partitions so a second matmul (lhsT=onehot_T[128p,10], rhs=g_T) can contract
over pixels, accumulating G[10,64] in PSUM across all blocks.

Phase B (per core): out tile [64,512] = means^T @ onehot[10,512] with the
means as stationary; means are split hi/lo into two bf16 matrices so two
accumulating bf16 matmuls reproduce fp32-accurate means (one-hot is exact
in bf16).
"""

import os
import numpy as np
import ml_dtypes

import concourse.bass as bass  # noqa: F401  (registers types)
import concourse.tile as tile
import concourse.bass_utils as bass_utils
from concourse import bacc, mybir

# Problem shape (hardcoded per contract)
B, CIN, COUT, H, W_ = 16, 128, 64, 192, 192
HW = H * W_                      # 36864
NB = 10                          # histogram bins
N_CORES = 8
BPC = B // N_CORES               # batches per core = 2
PPC = BPC * HW                   # pixels per core = 73728
BLK = 128                        # pixels per feature block (matmul stationary)
GROUP_PX = 1024                  # pixels per PSUM group = 8 blocks * 128
BLK_PER_GROUP = GROUP_PX // BLK  # 8
SLAB_PX = 4096                   # pixels per feature DMA slab
N_SLABS = PPC // SLAB_PX         # 18
GROUPS_PER_SLAB = SLAB_PX // GROUP_PX  # 4
N_GROUPS = PPC // GROUP_PX       # 72
N_BLOCKS = PPC // BLK            # 576
OHA_STRIDE = 16                  # onehot_T block stride (padded 10 -> 16)

BF16 = mybir.dt.bfloat16
F32 = mybir.dt.float32

_CACHE = {}

# exec times (ns) of the last kernel() call, per NEFF, when tracing enabled
LAST_EXEC_NS = {}


def _install_ntff_hook():
    """Optionally enable NTFF profiling under axon (agent image lacks
    antenv.axon_hooks). Best-effort; harmless if unavailable."""
    import sys, types
    if "antenv.axon_hooks" in sys.modules:
        return True
    try:
        mod = types.ModuleType("antenv.axon_hooks")
        _hook = [None]
        mod.set_axon_ntff_profile_hook = lambda h: _hook.__setitem__(0, h)
        mod.get_axon_ntff_profile_hook = lambda: _hook[0]
        import antenv
        from trn_agent_boot.trn_boot import _ntff_profile_via_ctypes
        antenv.axon_hooks = mod
        sys.modules["antenv.axon_hooks"] = mod
        mod.set_axon_ntff_profile_hook(
            _ntff_profile_via_ctypes("/opt/axon/libaxon_pjrt.so"))
        return True
    except Exception:
        return False


def _build_neff_a():
    """Phase A: per-core partial per-bin sums of conv outputs, G[10,64]."""
    nc = bacc.Bacc("TRN2", target_bir_lowering=False, debug=False,
                   enable_asserts=True, num_devices=N_CORES)
    feats_t = nc.dram_tensor("feats", [BPC, CIN, HW], F32, kind="ExternalInput")
    oha_t = nc.dram_tensor("oha", [128, N_BLOCKS * OHA_STRIDE], BF16,
                           kind="ExternalInput")
    wt_t = nc.dram_tensor("wt", [CIN, COUT], BF16, kind="ExternalInput")
    gpart_t = nc.dram_tensor("gpart", [NB, COUT], F32, kind="ExternalOutput")

    feats = feats_t.ap()
    with tile.TileContext(nc) as tc:
        with tc.tile_pool(name="cst", bufs=1) as cst, \
             tc.tile_pool(name="fpool", bufs=3) as fpool, \
             tc.tile_pool(name="gpool", bufs=3) as gpool, \
             tc.tile_pool(name="spool", bufs=1) as spool, \
             tc.tile_pool(name="pconv", bufs=3, space="PSUM") as pconv, \
             tc.tile_pool(name="pg", bufs=1, space="PSUM") as pg:

            wt_s = cst.tile([CIN, COUT], BF16)
            nc.sync.dma_start(wt_s[:], wt_t.ap()[:])
            oha_s = cst.tile([128, N_BLOCKS * OHA_STRIDE], BF16)
            nc.sync.dma_start(oha_s[:], oha_t.ap()[:])

            G_ps = pg.tile([NB, COUT], F32, space="PSUM")

            gi = 0
            for s in range(N_SLABS):
                px0 = s * SLAB_PX
                b, o = px0 // HW, px0 % HW
                fs = fpool.tile([CIN, SLAB_PX], BF16)
                # SWDGE cast f32 -> bf16 during the DMA
                nc.gpsimd.dma_start(fs[:], feats[b, :, o:o + SLAB_PX])
                for g in range(GROUPS_PER_SLAB):
                    ps = pconv.tile([128, 8 * COUT], F32, space="PSUM")
                    for j in range(BLK_PER_GROUP):
                        f0 = g * GROUP_PX + j * BLK
                        nc.tensor.matmul(
                            ps[:, COUT * j:COUT * (j + 1)],
                            lhsT=fs[:, f0:f0 + BLK],
                            rhs=wt_s[:],
                            start=True, stop=True)
                    gsb = gpool.tile([128, 8 * COUT], BF16)
                    if gi % 2 == 0:
                        nc.vector.tensor_copy(gsb[:], ps[:])
                    else:
                        nc.scalar.copy(gsb[:], ps[:])
                    for j in range(BLK_PER_GROUP):
                        blk = gi * BLK_PER_GROUP + j
                        nc.tensor.matmul(
                            G_ps[:],
                            lhsT=oha_s[:, blk * OHA_STRIDE:blk * OHA_STRIDE + NB],
                            rhs=gsb[:, COUT * j:COUT * (j + 1)],
                            start=(blk == 0), stop=(blk == N_BLOCKS - 1))
                    gi += 1

            g_out = spool.tile([NB, COUT], F32)
            nc.vector.tensor_copy(g_out[:], G_ps[:])
            nc.sync.dma_start(gpart_t.ap()[:], g_out[:])
    nc.compile()
    return nc


def _build_neff_b():
    """Phase B: out[b,:,p] = means[bin(p)] via a means-stationary matmul.

    The hi/lo bf16 split of means is fused into ONE K=20 matmul: stationary
    [mh; ml] [20,64], one-hot rows duplicated on partitions 10-19, PSUM
    accumulates both products in fp32.

    Output is staged in SBUF as [128=(half,chan), 4608] per 9216-pixel slab
    and written with a single big SWDGE DMA (2.36 MB) -- many small
    sync-ring DMAs serialize on one HWDGE queue at ~1/8 bandwidth.
    """
    nc = bacc.Bacc("TRN2", target_bir_lowering=False, debug=False,
                   enable_asserts=True, num_devices=N_CORES)
    mhl_t = nc.dram_tensor("mhl", [2 * NB, COUT], BF16, kind="ExternalInput")
    ohb_t = nc.dram_tensor("ohb", [2 * NB, PPC], BF16, kind="ExternalInput")
    out_t = nc.dram_tensor("out", [BPC, COUT, HW], F32, kind="ExternalOutput")

    OH_SLAB = 9216               # pixels per slab (36864 = 4 slabs per batch)
    HALF = OH_SLAB // 2          # 4608
    N_CH = HALF // 512           # 9 psum chunks per slab

    out_ap = out_t.ap()
    ohb = ohb_t.ap()
    with tile.TileContext(nc) as tc:
        with tc.tile_pool(name="cst", bufs=1) as cst, \
             tc.tile_pool(name="ohpool", bufs=2) as ohpool, \
             tc.tile_pool(name="stage", bufs=2) as stage, \
             tc.tile_pool(name="pout", bufs=4, space="PSUM") as pout:

            mhl_s = cst.tile([2 * NB, COUT], BF16)
            nc.sync.dma_start(mhl_s[:], mhl_t.ap()[:])

            ci = 0
            for s in range(PPC // OH_SLAB):
                px0 = s * OH_SLAB
                b, o = px0 // HW, px0 % HW
                oh_s = ohpool.tile([2 * NB, OH_SLAB], BF16)
                nc.sync.dma_start(oh_s[:], ohb[:, px0:px0 + OH_SLAB])
                st = stage.tile([128, HALF], F32)
                for u in range(N_CH):
                    po = pout.tile([128, 512], F32, space="PSUM")
                    # chunk u pairs pixels [512u,+512) (i=0 -> partitions
                    # 0-63) with [HALF+512u,+512) (i=1 -> partitions 64-127)
                    for i, colbase in ((0, 0), (1, 64)):
                        rhs = oh_s[:, i * HALF + u * 512:i * HALF + u * 512 + 512]
                        nc.tensor.matmul(po[colbase:colbase + 64, :],
                                         lhsT=mhl_s[:], rhs=rhs,
                                         start=True, stop=True,
                                         tile_position=(0, colbase))
                    if ci % 2 == 0:
                        nc.vector.tensor_copy(st[:, u * 512:u * 512 + 512], po[:])
                    else:
                        nc.scalar.copy(st[:, u * 512:u * 512 + 512], po[:])
                    ci += 1
                # one big SWDGE write: partition q=(i*64+c) holds pixels
                # [o+i*HALF, +HALF) of channel c
                dst = out_ap[b, :, o:o + OH_SLAB].rearrange("c (i p) -> i c p",
                                                            i=2)
                nc.gpsimd.dma_start(dst, st[:])
    nc.compile()
    return nc


def _get_modules():
    if "a" not in _CACHE:
        _CACHE["a"] = _build_neff_a()
        _CACHE["b"] = _build_neff_b()
    return _CACHE["a"], _CACHE["b"]


def kernel(features, depth, weight, bias, depthpool=None):
    trace = bool(int(os.environ.get("KERNEL_TRACE", "0")))
    if trace:
        trace = _install_ntff_hook()

    features = np.asarray(features, dtype=np.float32)
    depth = np.asarray(depth, dtype=np.float32)
    weight = np.asarray(weight, dtype=np.float32)
    bias = np.asarray(bias, dtype=np.float32)

    # ---- host: histogram binning of depth (exact f32 replica of reference)
    d = depth[:, 0]                                     # [B, H, W] f32
    dmin, dmax = d.min(), d.max()
    width = np.float32((dmax - dmin) / np.float32(NB))
    bins = np.clip(np.floor((d - dmin) / width).astype(np.int32), 0, NB - 1)
    bins = bins.reshape(B, HW)
    counts = np.bincount(bins.ravel(), minlength=NB).astype(np.float64)

    arange_nb = np.arange(NB, dtype=np.int32)
    wt_bf = np.ascontiguousarray(weight.T).astype(ml_dtypes.bfloat16)  # [128,64]

    in_maps_a = []
    in_maps_b_onehot = []
    for c in range(N_CORES):
        binsc = bins[BPC * c:BPC * (c + 1)].reshape(PPC)       # [73728]
        # onehot_T, padded: oha[p, blk*16 + n] = (binsc[blk*128+p] == n)
        bb = binsc.reshape(N_BLOCKS, BLK)                       # [576, 128]
        oha = np.zeros((128, N_BLOCKS, OHA_STRIDE), dtype=ml_dtypes.bfloat16)
        oha[:, :, :NB] = (bb.T[:, :, None] == arange_nb).astype(ml_dtypes.bfloat16)
        oha = np.ascontiguousarray(oha.reshape(128, N_BLOCKS * OHA_STRIDE))
        # onehot rows, duplicated for the fused hi/lo matmul:
        # ohb[n, p] = ohb[10+n, p] = (binsc[p] == n)
        oh1 = (arange_nb[:, None] == binsc[None, :]).astype(ml_dtypes.bfloat16)
        ohb = np.ascontiguousarray(np.concatenate([oh1, oh1], axis=0))
        feats_c = features[BPC * c:BPC * (c + 1)].reshape(BPC, CIN, HW)
        in_maps_a.append({"feats": feats_c, "oha": oha, "wt": wt_bf})
        in_maps_b_onehot.append(ohb)

    nc_a, nc_b = _get_modules()
    core_ids = list(range(N_CORES))

    res_a = bass_utils.run_bass_kernel_spmd(nc_a, in_maps_a, core_ids=core_ids,
                                            trace=trace)
    if trace:
        LAST_EXEC_NS["A"] = res_a.exec_time_ns

    G = np.zeros((NB, COUT), dtype=np.float64)
    for c in range(N_CORES):
        G += res_a.results[c]["gpart"].astype(np.float64)

    means = G / np.maximum(counts, 1.0)[:, None] \
        + bias.astype(np.float64)[None, :] * (counts > 0)[:, None]
    means = means.astype(np.float32)
    mh = means.astype(ml_dtypes.bfloat16)
    ml = (means - mh.astype(np.float32)).astype(ml_dtypes.bfloat16)
    mhl = np.ascontiguousarray(np.concatenate([mh, ml], axis=0))  # [20, 64]

    in_maps_b = [{"mhl": mhl, "ohb": in_maps_b_onehot[c]}
                 for c in range(N_CORES)]
    res_b = bass_utils.run_bass_kernel_spmd(nc_b, in_maps_b, core_ids=core_ids,
                                            trace=trace)
    if trace:
        LAST_EXEC_NS["B"] = res_b.exec_time_ns

    out = np.empty((B, COUT, H, W_), dtype=np.float32)
    for c in range(N_CORES):
        out[BPC * c:BPC * (c + 1)] = \
            res_b.results[c]["out"].reshape(BPC, COUT, H, W_)
    return out


# revision 9
# speedup vs baseline: 1.8419x; 1.3150x over previous
"""Trainium2 Bass kernel for nn_DepthGlobalPool (histogram_binning).

Math: out[b,:,h,w] = means[bin(b,h,w)] where
  bin = histogram bin of depth over global [min,max], 10 equal bins
  means[n] = mean over pixels p in bin n of (W @ features[p] + bias)
Because the 1x1 conv is linear, the per-bin sums of conv outputs equal
W @ (per-bin sums of features) + count*bias, so the per-pixel conv never
needs to be materialized:
  G[n, o]  = sum_{p in bin n} (W @ features[p])[o]      (device, phase A)
  means    = G_global / max(counts,1) + bias*(counts>0) (host, tiny)
  out[p]   = means[bin(p)]                              (device, phase B)

Distribution: data-parallel over batch B (2 batches per core on 8 cores).
Phase A produces per-core partial G [10,64]; the tiny partials are reduced
on host between the two NEFF launches (cheaper + more deterministic than an
on-device AllReduce, which measured 35-70us of latency+skew).

Phase A (per core): for each 128-pixel block, matmul with the feature block
as the STATIONARY operand (lhsT=[128c,128p], rhs=W^T[128c,64]) produces the
conv output transposed, g_T[128p,64], in PSUM -- this puts pixels on
partitions so a second matmul (lhsT=onehot_T[128p,10], rhs=g_T) can contract
over pixels, accumulating G[10,64] in PSUM across all blocks.

Phase B (per core): out tile [64,512] = means^T @ onehot[10,512] with the
means as stationary; means are split hi/lo into two bf16 matrices so two
accumulating bf16 matmuls reproduce fp32-accurate means (one-hot is exact
in bf16).
"""

import os
import numpy as np
import ml_dtypes

import concourse.bass as bass  # noqa: F401  (registers types)
import concourse.tile as tile
import concourse.bass_utils as bass_utils
from concourse import bacc, mybir

# Problem shape (hardcoded per contract)
B, CIN, COUT, H, W_ = 16, 128, 64, 192, 192
HW = H * W_                      # 36864
NB = 10                          # histogram bins
N_CORES = 8
BPC = B // N_CORES               # batches per core = 2
PPC = BPC * HW                   # pixels per core = 73728
BLK = 128                        # pixels per feature block (matmul stationary)
GROUP_PX = 1024                  # pixels per PSUM group = 8 blocks * 128
BLK_PER_GROUP = GROUP_PX // BLK  # 8
SLAB_PX = 4096                   # pixels per feature DMA slab
N_SLABS = PPC // SLAB_PX         # 18
GROUPS_PER_SLAB = SLAB_PX // GROUP_PX  # 4
N_GROUPS = PPC // GROUP_PX       # 72
N_BLOCKS = PPC // BLK            # 576
OHA_STRIDE = 16                  # onehot_T block stride (padded 10 -> 16)

BF16 = mybir.dt.bfloat16
F32 = mybir.dt.float32

_CACHE = {}

# exec times (ns) of the last kernel() call, per NEFF, when tracing enabled
LAST_EXEC_NS = {}


def _install_ntff_hook():
    """Optionally enable NTFF profiling under axon (agent image lacks
    antenv.axon_hooks). Best-effort; harmless if unavailable."""
    import sys, types
    if "antenv.axon_hooks" in sys.modules:
        return True
    try:
        mod = types.ModuleType("antenv.axon_hooks")
        _hook = [None]
        mod.set_axon_ntff_profile_hook = lambda h: _hook.__setitem__(0, h)
        mod.get_axon_ntff_profile_hook = lambda: _hook[0]
        import antenv
        from trn_agent_boot.trn_boot import _ntff_profile_via_ctypes
        antenv.axon_hooks = mod
        sys.modules["antenv.axon_hooks"] = mod
        mod.set_axon_ntff_profile_hook(
            _ntff_profile_via_ctypes("/opt/axon/libaxon_pjrt.so"))
        return True
    except Exception:
        return False


def _build_neff_a():
    """Phase A: per-core partial per-bin sums of conv outputs, G[10,64]."""
    nc = bacc.Bacc("TRN2", target_bir_lowering=False, debug=False,
                   enable_asserts=True, num_devices=N_CORES)
    feats_t = nc.dram_tensor("feats", [BPC, CIN, HW], F32, kind="ExternalInput")
    oha_t = nc.dram_tensor("oha", [128, N_BLOCKS * OHA_STRIDE], BF16,
                           kind="ExternalInput")
    wt_t = nc.dram_tensor("wt", [CIN, COUT], BF16, kind="ExternalInput")
    gpart_t = nc.dram_tensor("gpart", [NB, COUT], F32, kind="ExternalOutput")

    feats = feats_t.ap()
    with tile.TileContext(nc) as tc:
        with tc.tile_pool(name="cst", bufs=1) as cst, \
             tc.tile_pool(name="fpool", bufs=3) as fpool, \
             tc.tile_pool(name="gpool", bufs=3) as gpool, \
             tc.tile_pool(name="spool", bufs=1) as spool, \
             tc.tile_pool(name="pconv", bufs=3, space="PSUM") as pconv, \
             tc.tile_pool(name="pg", bufs=1, space="PSUM") as pg:

            wt_s = cst.tile([CIN, COUT], BF16)
            nc.sync.dma_start(wt_s[:], wt_t.ap()[:])
            oha_s = cst.tile([128, N_BLOCKS * OHA_STRIDE], BF16)
            nc.sync.dma_start(oha_s[:], oha_t.ap()[:])

            G_ps = pg.tile([NB, COUT], F32, space="PSUM")

            gi = 0
            for s in range(N_SLABS):
                px0 = s * SLAB_PX
                b, o = px0 // HW, px0 % HW
                fs = fpool.tile([CIN, SLAB_PX], BF16)
                # SWDGE cast f32 -> bf16 during the DMA
                nc.gpsimd.dma_start(fs[:], feats[b, :, o:o + SLAB_PX])
                for g in range(GROUPS_PER_SLAB):
                    ps = pconv.tile([128, 8 * COUT], F32, space="PSUM")
                    for j in range(BLK_PER_GROUP):
                        f0 = g * GROUP_PX + j * BLK
                        nc.tensor.matmul(
                            ps[:, COUT * j:COUT * (j + 1)],
                            lhsT=fs[:, f0:f0 + BLK],
                            rhs=wt_s[:],
                            start=True, stop=True)
                    gsb = gpool.tile([128, 8 * COUT], BF16)
                    if gi % 2 == 0:
                        nc.vector.tensor_copy(gsb[:], ps[:])
                    else:
                        nc.scalar.copy(gsb[:], ps[:])
                    for j in range(BLK_PER_GROUP):
                        blk = gi * BLK_PER_GROUP + j
                        nc.tensor.matmul(
                            G_ps[:],
                            lhsT=oha_s[:, blk * OHA_STRIDE:blk * OHA_STRIDE + NB],
                            rhs=gsb[:, COUT * j:COUT * (j + 1)],
                            start=(blk == 0), stop=(blk == N_BLOCKS - 1))
                    gi += 1

            g_out = spool.tile([NB, COUT], F32)
            nc.vector.tensor_copy(g_out[:], G_ps[:])
            nc.sync.dma_start(gpart_t.ap()[:], g_out[:])
    nc.compile()
    return nc


def _build_neff_b():
    """Phase B: out[b,:,p] = means[bin(p)] via a means-stationary matmul.

    The hi/lo bf16 split of means is fused into ONE K=20 matmul per 512-px
    chunk: stationary [mh; ml] [20,64], one-hot rows duplicated for the lo
    half, PSUM accumulates both products in fp32.

    DMA-width tricks (both streams must use all 128 partitions to get
    full HBM bandwidth):
      * one-hot is packed [128, PPC/4]: partition rows 32g..32g+20 hold the
        (duplicated) one-hot of the g-th QUARTER of this core's pixels.
        The stationary is replicated at partitions 32g too, since matmul
        requires lhsT/rhs to share a base partition (explicit
        tile_position=(32g, colbase)).
      * output is staged in SBUF as [128=(half,chan), 4608] per 9216-pixel
        slab and written with one 2.36 MB SWDGE DMA (many small sync-ring
        DMAs serialize on one HWDGE queue at ~1/8 bandwidth).
    """
    nc = bacc.Bacc("TRN2", target_bir_lowering=False, debug=False,
                   enable_asserts=True, num_devices=N_CORES)
    mhl_t = nc.dram_tensor("mhl", [128, COUT], BF16, kind="ExternalInput")
    ohb_t = nc.dram_tensor("ohb", [128, PPC // 4], BF16, kind="ExternalInput")
    out_t = nc.dram_tensor("out", [BPC, COUT, HW], F32, kind="ExternalOutput")

    OH_SLAB = 9216               # output pixels per slab (36864 = 4 per batch)
    HALF = OH_SLAB // 2          # 4608
    N_CH = HALF // 512           # 9 psum chunks per slab
    QUARTER = PPC // 4           # 18432 pixels per one-hot partition group

    out_ap = out_t.ap()
    ohb = ohb_t.ap()
    with tile.TileContext(nc) as tc:
        with tc.tile_pool(name="cst", bufs=1) as cst, \
             tc.tile_pool(name="ohpool", bufs=2) as ohpool, \
             tc.tile_pool(name="stage", bufs=3) as stage, \
             tc.tile_pool(name="pout", bufs=4, space="PSUM") as pout:

            mhl_s = cst.tile([128, COUT], BF16)
            nc.sync.dma_start(mhl_s[:], mhl_t.ap()[:])

            ci = 0
            for cs in range(2):      # one-hot column-slab: quarter halves
                oh_s = ohpool.tile([128, OH_SLAB], BF16)
                nc.sync.dma_start(oh_s[:], ohb[:, cs * OH_SLAB:(cs + 1) * OH_SLAB])
                for g in range(4):   # pixel quarter -> output slab 2g+cs
                    px0 = g * QUARTER + cs * OH_SLAB
                    b, o = px0 // HW, px0 % HW
                    st = stage.tile([128, HALF], F32)
                    lhs = mhl_s[32 * g:32 * g + 2 * NB, :]
                    for u in range(N_CH):
                        po = pout.tile([128, 512], F32, space="PSUM")
                        # chunk u pairs pixels [px0+512u,+512) (i=0 ->
                        # partitions 0-63) with [px0+HALF+512u,+512) (i=1)
                        for i, colbase in ((0, 0), (1, 64)):
                            j0 = i * HALF + u * 512
                            rhs = oh_s[32 * g:32 * g + 2 * NB, j0:j0 + 512]
                            nc.tensor.matmul(po[colbase:colbase + 64, :],
                                             lhsT=lhs, rhs=rhs,
                                             start=True, stop=True,
                                             tile_position=(32 * g, colbase))
                        if ci % 2 == 0:
                            nc.vector.tensor_copy(st[:, u * 512:u * 512 + 512],
                                                  po[:])
                        else:
                            nc.scalar.copy(st[:, u * 512:u * 512 + 512], po[:])
                        ci += 1
                    # partition q=(i*64+c) holds pixels [o+i*HALF,+HALF) of
                    # channel c
                    dst = out_ap[b, :, o:o + OH_SLAB].rearrange(
                        "c (i p) -> i c p", i=2)
                    nc.gpsimd.dma_start(dst, st[:])
    nc.compile()
    return nc


def _get_modules():
    if "a" not in _CACHE:
        _CACHE["a"] = _build_neff_a()
        _CACHE["b"] = _build_neff_b()
    return _CACHE["a"], _CACHE["b"]


def kernel(features, depth, weight, bias, depthpool=None):
    trace = bool(int(os.environ.get("KERNEL_TRACE", "0")))
    if trace:
        trace = _install_ntff_hook()

    features = np.asarray(features, dtype=np.float32)
    depth = np.asarray(depth, dtype=np.float32)
    weight = np.asarray(weight, dtype=np.float32)
    bias = np.asarray(bias, dtype=np.float32)

    # ---- host: histogram binning of depth (exact f32 replica of reference)
    d = depth[:, 0]                                     # [B, H, W] f32
    dmin, dmax = d.min(), d.max()
    width = np.float32((dmax - dmin) / np.float32(NB))
    bins = np.clip(np.floor((d - dmin) / width).astype(np.int32), 0, NB - 1)
    bins = bins.reshape(B, HW)
    counts = np.bincount(bins.ravel(), minlength=NB).astype(np.float64)

    arange_nb = np.arange(NB, dtype=np.int32)
    wt_bf = np.ascontiguousarray(weight.T).astype(ml_dtypes.bfloat16)  # [128,64]

    in_maps_a = []
    in_maps_b_onehot = []
    for c in range(N_CORES):
        binsc = bins[BPC * c:BPC * (c + 1)].reshape(PPC)       # [73728]
        # onehot_T, padded: oha[p, blk*16 + n] = (binsc[blk*128+p] == n)
        bb = binsc.reshape(N_BLOCKS, BLK)                       # [576, 128]
        oha = np.zeros((128, N_BLOCKS, OHA_STRIDE), dtype=ml_dtypes.bfloat16)
        oha[:, :, :NB] = (bb.T[:, :, None] == arange_nb).astype(ml_dtypes.bfloat16)
        oha = np.ascontiguousarray(oha.reshape(128, N_BLOCKS * OHA_STRIDE))
        # one-hot packed [128, PPC/4]: rows 32g+n and 32g+10+n hold
        # (bins[g*QUARTER + j] == n); rows 32g+20..31 stay zero
        quarter = PPC // 4
        ohb = np.zeros((128, quarter), dtype=ml_dtypes.bfloat16)
        for g in range(4):
            oh1 = (arange_nb[:, None] ==
                   binsc[None, g * quarter:(g + 1) * quarter]
                   ).astype(ml_dtypes.bfloat16)
            ohb[32 * g:32 * g + NB] = oh1
            ohb[32 * g + NB:32 * g + 2 * NB] = oh1
        feats_c = features[BPC * c:BPC * (c + 1)].reshape(BPC, CIN, HW)
        in_maps_a.append({"feats": feats_c, "oha": oha, "wt": wt_bf})
        in_maps_b_onehot.append(ohb)

    nc_a, nc_b = _get_modules()
    core_ids = list(range(N_CORES))

    res_a = bass_utils.run_bass_kernel_spmd(nc_a, in_maps_a, core_ids=core_ids,
                                            trace=trace)
    if trace:
        LAST_EXEC_NS["A"] = res_a.exec_time_ns

    G = np.zeros((NB, COUT), dtype=np.float64)
    for c in range(N_CORES):
        G += res_a.results[c]["gpart"].astype(np.float64)

    means = G / np.maximum(counts, 1.0)[:, None] \
        + bias.astype(np.float64)[None, :] * (counts > 0)[:, None]
    means = means.astype(np.float32)
    mh = means.astype(ml_dtypes.bfloat16)
    ml = (means - mh.astype(np.float32)).astype(ml_dtypes.bfloat16)
    # stationary [mh; ml] replicated at partition bases 0/32/64/96
    mhl = np.zeros((128, COUT), dtype=ml_dtypes.bfloat16)
    for g in range(4):
        mhl[32 * g:32 * g + NB] = mh
        mhl[32 * g + NB:32 * g + 2 * NB] = ml

    in_maps_b = [{"mhl": mhl, "ohb": in_maps_b_onehot[c]}
                 for c in range(N_CORES)]
    res_b = bass_utils.run_bass_kernel_spmd(nc_b, in_maps_b, core_ids=core_ids,
                                            trace=trace)
    if trace:
        LAST_EXEC_NS["B"] = res_b.exec_time_ns

    out = np.empty((B, COUT, H, W_), dtype=np.float32)
    for c in range(N_CORES):
        out[BPC * c:BPC * (c + 1)] = \
            res_b.results[c]["out"].reshape(BPC, COUT, H, W_)
    return out
